# revision 31
# baseline (speedup 1.0000x reference)
"""Trainium2 Bass kernel for nn_ExplicitSVDBlock (dense transformer block).

Sharding: 8 NeuronCores = 4 batches x 2 query-halves of 1024 tokens.
Each core receives its batch's full 2048 tokens (permuted so its own
query tokens come first), redundantly builds K/V for all keys, and
computes everything else for its 1024 query tokens.  Zero cross-core
communication; host gathers the 8 [1024, 768] shards.

Device program: feature-major activations for matmuls (PE transposes
bridge to token-major for layernorm/residual), float32r matmul dtype,
softmax via exp on ScalarE with a [V | 1]-augmented stationary so the
denominators come out of the same PE accumulation.

I/O: all tensors are packed host-side into two DRAM blobs — a per-core
`xin` ([128, 16384]: x blocks + cos/sin tables) and a core-invariant
weight blob `wb` ([128, 66888]) — so each dispatch binds 2 input
buffers + 1 output instead of 27.  The axon per-dispatch cost is
~26 us per bound input buffer, which dominated the old launch path.
"""
import sys

if '/opt/trn_rl_repo' not in sys.path:
    sys.path.insert(0, '/opt/trn_rl_repo')

import numpy as np
import concourse.bass as bass
import concourse.bacc as bacc
import concourse.mybir as mybir
import concourse.tile as tile
from concourse.bass_utils import run_bass_kernel_spmd
from concourse.masks import make_identity

F32 = mybir.dt.float32
F32R = mybir.dt.float32r
BF16 = mybir.dt.bfloat16
AF = mybir.ActivationFunctionType
OP = mybir.AluOpType

B, S, D, H, HD, RA = 4, 2048, 768, 12, 64, 32
RF, DFF = 512, 3072
P = 128
SK, SQ = S, S // 2          # keys per core / queries per core
HRA = H * RA                # 384
MT_D = D // P               # 6
KT_A = HRA // P             # 3
NKT = SK // P               # 16
NQT = SQ // P               # 8
QCH = 256                   # attention query chunk
NQC = SQ // QCH
KB = 4                      # score k-tiles per exp batch
MT_RF = RF // P             # 4
MT_DFF = DFF // P           # 24
NDCH = DFF // 512           # 6
TCH = 256                   # build token chunk
TCH3 = 512                  # post-attention token chunk
SKH = SK // 2
LN_EPS = 1e-6
N_CORES = 8

# ---- packed input layouts ----
_XL, _WL = {}, {}


def _mk_layout():
    xo = 0
    for name, n in (("xfull", NKT * D), ("cos2", SK), ("sin2", SK)):
        _XL[name] = (xo, n)
        xo += n
    wo = 0
    ents = [("ucat_q", KT_A * HRA * 2), ("ucat_k", KT_A * HRA * 2),
            ("ucat_v", KT_A * HRA * 2),
            ("bdv_q", MT_D * P), ("bdv_qr", MT_D * P),
            ("bdv_k", MT_D * P), ("bdv_kr", MT_D * P),
            ("bias_q", MT_D), ("bias_qr", MT_D),
            ("bias_k", MT_D), ("bias_kr", MT_D),
            ("bdvv", KT_A * D), ("bv", D), ("wot", H * D), ("wo_b", D),
            ("ui", MT_D * RF), ("vi", NDCH * 2 * MT_RF * 512),
            ("bi1t", MT_DFF), ("bi2t", MT_DFF),
            ("uo", NDCH * MT_RF * 512), ("vo", MT_RF * D), ("bo", D)]
    for name, n in ents:
        _WL[name] = (wo, n)
        wo += n
    return xo, wo


XCOLS, WCOLS = _mk_layout()

_CACHE = {}
import os
_PHASES = int(os.environ.get("BASS_KERNEL_PHASES", "4"))
_BCAST_MM = int(os.environ.get("BASS_BCAST_MM", "1"))


def _declare_io(nc):
    t = {}
    t["xin"] = nc.dram_tensor("xin", [P, XCOLS], F32, kind="ExternalInput")
    t["wb"] = nc.dram_tensor("wb", [P, WCOLS], F32, kind="ExternalInput")
    t["out"] = nc.dram_tensor("out", [SQ, D], F32, kind="ExternalOutput")
    if not _BCAST_MM:
        t["nrm"] = nc.dram_tensor("nrm_scratch", [H, NQC, QCH], F32)  # internal
    return t


def _emit(nc, tc, t):
    rsc = float(1.0 / np.sqrt(HD))

    def wap(name, off=0, n=None):
        base, tot = _WL[name]
        if n is None:
            n = tot
        return t["wb"].ap()[:, base + off:base + off + n]

    def xap(name, off=0, n=None):
        base, tot = _XL[name]
        if n is None:
            n = tot
        return t["xin"].ap()[:, base + off:base + off + n]

    def xblk(i):
        # 128-token block i of this core's permuted x, as [P, D]
        return xap("xfull", i * D, D)

    const_cm = tc.tile_pool(name="const", bufs=1)
    const = const_cm.__enter__()
    ident = const.tile([P, P], F32)
    make_identity(nc, ident)
    # one-hot-row stationary for broadcasting the softmax recip row: rows
    # 64..127 are the 64-contract lanes, only lane 0 (partition 64) is 1.0
    bsel = const.tile([P, 64], F32)
    nc.vector.memset(bsel[:], 0.0)
    nc.vector.memset(bsel[HD:HD + 1, :], 1.0)

    poolQKV_cm = tc.tile_pool(name="pQKV", bufs=1)
    poolQKV = poolQKV_cm.__enter__()
    qTr = poolQKV.tile([P, MT_D, SQ], F32R)
    kTr = poolQKV.tile([P, MT_D, SK], F32R)
    vaug = poolQKV.tile([P, NKT, H * (HD + 1)], F32R)
    vaug4 = vaug[:].rearrange("p n (h e) -> p n h e", h=H)

    # ---- phase 1: LN1 + QKV build ----
    with tc.tile_pool(name="bw", bufs=1) as wpool, \
         tc.tile_pool(name="bh", bufs=2) as hpool, \
         tc.tile_pool(name="bxu", bufs=2) as xupool, \
         tc.tile_pool(name="brot", bufs=1) as rotpool, \
         tc.tile_pool(name="bx", bufs=2) as xpool, \
         tc.tile_pool(name="bst", bufs=3) as stpool, \
         tc.tile_pool(name="psA", bufs=3, space="PSUM") as psA, \
         tc.tile_pool(name="psB", bufs=2, space="PSUM") as psB, \
         tc.tile_pool(name="psV", bufs=1, space="PSUM") as psV:

        ucat, bdv, bias = {}, {}, {}
        for p in ("q", "k", "v"):
            w = wpool.tile([P, MT_D, HRA], F32R, tag=f"ucat_{p}")
            nc.sync.dma_start(w[:], wap(f"ucat_{p}").rearrange(
                "p (k m) -> p k m", m=HRA).bitcast(F32R))
            ucat[p] = w
        for p in ("q", "qr", "k", "kr"):
            w = wpool.tile([P, MT_D, P], F32R, tag=f"bdv_{p}")
            nc.sync.dma_start(w[:], wap(f"bdv_{p}").rearrange(
                "p (m x) -> p m x", x=P).bitcast(F32R))
            bdv[p] = w
            bl = wpool.tile([P, MT_D], F32, tag=f"bias_{p}")
            nc.sync.dma_start(bl[:], wap(f"bias_{p}"))
            bias[p] = bl
        bdvv = wpool.tile([P, KT_A, D], F32R)
        nc.sync.dma_start(bdvv[:], wap("bdvv").rearrange(
            "p (k d) -> p k d", d=D).bitcast(F32R))
        bv_bc = wpool.tile([P, D], F32)
        nc.sync.dma_start(bv_bc[:], wap("bv"))
        eps_t = wpool.tile([P, 1], F32)
        nc.vector.memset(eps_t[:], LN_EPS)
        ones_h = wpool.tile([P, H], F32)
        nc.vector.memset(ones_h[:], 1.0)
        for _kt in range(NKT):
            nc.vector.tensor_copy(vaug4[:, _kt, :, HD], ones_h[:])

        for half in range(2):
            goff = half * SKH
            for tch in range(SKH // TCH):
                coff = tch * TCH
                gcoff = goff + coff
                hT = hpool.tile([P, MT_D, TCH], F32R, tag="hT")
                cosc = hpool.tile([P, TCH], F32, tag="cosc")
                sinc = hpool.tile([P, TCH], F32, tag="sinc")
                nc.sync.dma_start(cosc[:], xap("cos2", gcoff, TCH))
                nc.sync.dma_start(sinc[:], xap("sin2", gcoff, TCH))

                for tb in range(TCH // P):
                    x_t = xpool.tile([P, D], F32, tag="x_t")
                    r0 = gcoff + tb * P
                    nc.sync.dma_start(x_t[:], xblk(r0 // P))
                    xg = x_t[:].rearrange("p (n s) -> p n s", s=256)
                    stats = stpool.tile([P, D // 256, 6], F32, tag="stats")
                    for g in range(D // 256):
                        nc.vector.bn_stats(stats[:, g, :], xg[:, g, :])
                    mv = stpool.tile([P, 2], F32, tag="mv")
                    nc.vector.bn_aggr(mv[:], stats[:])
                    rstd = stpool.tile([P, 1], F32, tag="rstd")
                    nc.scalar.activation(rstd[:], mv[:, 1:2], AF.Sqrt, bias=eps_t[:])
                    nc.vector.reciprocal(rstd[:], rstd[:])
                    nc.vector.tensor_scalar(x_t[:], x_t[:], mv[:, 0:1], rstd[:],
                                            OP.subtract, OP.mult)
                    for mg in range(MT_D // 3):
                        ps = psA.tile([P, 3, P], F32, tag="b1")
                        for j in range(3):
                            mt = mg * 3 + j
                            nc.tensor.transpose(ps[:, j, :],
                                                x_t[:, mt * P:(mt + 1) * P], ident[:])
                        nc.scalar.activation(
                            hT[:, mg * 3:(mg + 1) * 3, tb * P:(tb + 1) * P],
                            ps[:], AF.Copy)

                projs = ["k", "v"] + (["q"] if half == 0 else [])
                for p in projs:
                    xs = xupool.tile([P, KT_A, TCH], F32R, tag="xu_sb")
                    for ma in range(KT_A):
                        xps = psA.tile([P, TCH], F32, tag="b1")
                        for kt in range(MT_D):
                            nc.tensor.matmul(xps[:], ucat[p][:, kt, ma * P:(ma + 1) * P],
                                             hT[:, kt, :],
                                             start=(kt == 0), stop=(kt == MT_D - 1))
                        nc.scalar.activation(xs[:, ma, :], xps[:], AF.Copy)
                    if p == "v":
                        for tb in range(TCH // P):
                            vps = psV.tile([P, D], F32, tag="v_ps")
                            for n0 in range(0, D, 512):
                                n1 = min(n0 + 512, D)
                                for ka in range(KT_A):
                                    nc.tensor.matmul(vps[:, n0:n1],
                                                     xs[:, ka, tb * P:(tb + 1) * P],
                                                     bdvv[:, ka, n0:n1],
                                                     start=(ka == 0),
                                                     stop=(ka == KT_A - 1))
                            ktg = gcoff // P + tb
                            nc.vector.tensor_tensor(
                                vaug4[:, ktg, :, 0:HD],
                                vps[:].rearrange("p (h e) -> p h e", h=H),
                                bv_bc[:].rearrange("p (h e) -> p h e", h=H),
                                OP.add)
                    else:
                        dst = qTr if p == "q" else kTr
                        dcols = slice(coff, coff + TCH) if p == "q" else \
                                slice(gcoff, gcoff + TCH)
                        rot = rotpool.tile([P, MT_D, TCH], F32R, tag="rot")
                        for m in range(MT_D):
                            ps2 = psB.tile([P, TCH], F32, tag="st2")
                            nc.tensor.matmul(ps2[:], bdv[p][:, m, :], xs[:, m // 2, :],
                                             start=True, stop=True)
                            nc.scalar.activation(dst[:, m, dcols], ps2[:], AF.Identity,
                                                 bias=bias[p][:, m:m + 1])
                            ps3 = psB.tile([P, TCH], F32, tag="st2")
                            nc.tensor.matmul(ps3[:], bdv[p + "r"][:, m, :],
                                             xs[:, m // 2, :], start=True, stop=True)
                            nc.vector.scalar_tensor_tensor(
                                rot[:, m, :], ps3[:], bias[p + "r"][:, m:m + 1],
                                sinc[:], OP.add, OP.mult)
                        dsl = dst[:, :, dcols]
                        cb = cosc[:, None, :].to_broadcast([P, MT_D, TCH])
                        nc.vector.tensor_tensor(dsl, dsl, cb, OP.mult)
                        nc.vector.tensor_tensor(dsl, dsl, rot[:], OP.add)

    # ---- phase 2: attention ----
    if _PHASES < 2:
        poolQKV_cm.__exit__(None, None, None)
        with tc.tile_pool(name="fb", bufs=2) as fb:
            for tt in range(NQT):
                ft = fb.tile([P, D], F32, tag="ft")
                nc.sync.dma_start(ft[:], xblk(tt))
                nc.sync.dma_start(t["out"][tt * P:(tt + 1) * P, :], ft[:])
        const_cm.__exit__(None, None, None)
        return
    poolO_cm = tc.tile_pool(name="pO", bufs=1, side="right")
    poolO = poolO_cm.__enter__()
    oTn = poolO.tile([64, H, SQ], F32R)
    if _BCAST_MM:
        srow_p = poolO.tile([P, QCH], F32)
        nc.vector.memset(srow_p[:], 0.0)

    with tc.tile_pool(name="aexp", bufs=2, side="right") as apool, \
         tc.tile_pool(name="anrm", bufs=3, side="right") as npool, \
         tc.tile_pool(name="psS", bufs=3, space="PSUM") as psS, \
         tc.tile_pool(name="psO", bufs=2, space="PSUM") as psO:
        items = [(h, qc) for h in range(H) for qc in range(NQC)]

        def emit_scores(i):
            h, qc = items[i]
            pair, hh = h // 2, h % 2
            rs = slice(hh * 64, hh * 64 + 64)
            qcols = slice(qc * QCH, (qc + 1) * QCH)
            expS = apool.tile([P, NKT, QCH], F32R, tag="expS")
            for kb in range(NKT // KB):
                sps = psS.tile([P, KB, QCH], F32, tag="s_ps")
                for j in range(KB):
                    kt = kb * KB + j
                    nc.tensor.matmul(sps[:, j, :],
                                     kTr[rs, pair, kt * P:(kt + 1) * P],
                                     qTr[rs, pair, qcols],
                                     start=True, stop=True)
                nc.scalar.activation(expS[:, kb * KB:(kb + 1) * KB, :],
                                     sps[:], AF.Exp, scale=rsc)
            return expS

        def emit_av(i, expS):
            h, qc = items[i]
            qcols = slice(qc * QCH, (qc + 1) * QCH)
            po = psO.tile([P, QCH], F32, tag="o_ps")
            for kt in range(NKT):
                nc.tensor.matmul(po[0:HD + 1, :], vaug4[:, kt, h, :],
                                 expS[:, kt, :],
                                 start=(kt == 0), stop=(kt == NKT - 1))
            if _BCAST_MM:
                nc.vector.reciprocal(srow_p[HD:HD + 1, :], po[HD:HD + 1, :])
                # broadcast the recip row across the 64 o-lanes on the PE
                # (one-hot-row stationary) instead of a DRAM roundtrip
                rb = psS.tile([P, KB, QCH], F32, tag="s_ps")
                nc.tensor.matmul(rb[0:64, 0, :], bsel[HD:, :], srow_p[HD:, :],
                                 start=True, stop=True)
                # DVE cannot read two PSUM operands; stage rb through SBUF
                rbs = npool.tile([64, QCH], F32, tag="rbs")
                nc.vector.tensor_copy(rbs[:], rb[0:64, 0, :])
                nc.vector.tensor_tensor(oTn[:, h, qcols], po[0:HD, :],
                                        rbs[:], OP.mult)
            else:
                srow = npool.tile([P, QCH], F32, tag="srow")
                nc.vector.reciprocal(srow[HD:HD + 1, :], po[HD:HD + 1, :])
                nc.sync.dma_start(t["nrm"][h, qc, :], srow[HD:HD + 1, :])
                rbd = npool.tile([64, QCH], F32, tag="rb")
                nc.gpsimd.dma_start(
                    rbd[:], bass.AP(t["nrm"].ap().tensor,
                                    (h * NQC + qc) * QCH, [[0, 64], [1, QCH]]))
                nc.vector.tensor_tensor(oTn[:, h, qcols], po[0:HD, :],
                                        rbd[:], OP.mult)

        # software pipeline: scores for item i+1 overlap AV/normalize of item i
        prev = None
        for i in range(len(items) + 1):
            cur = emit_scores(i) if i < len(items) else None
            if prev is not None:
                emit_av(i - 1, prev)
            prev = cur
    poolQKV_cm.__exit__(None, None, None)

    # ---- phase 3: Wo + residual + LN2 ----
    if _PHASES < 3:
        poolO_cm.__exit__(None, None, None)
        with tc.tile_pool(name="fb", bufs=2) as fb:
            for tt in range(NQT):
                ft = fb.tile([P, D], F32, tag="ft")
                nc.sync.dma_start(ft[:], xblk(tt))
                nc.sync.dma_start(t["out"][tt * P:(tt + 1) * P, :], ft[:])
        const_cm.__exit__(None, None, None)
        return
    poolX_cm = tc.tile_pool(name="pX", bufs=1)
    poolX = poolX_cm.__enter__()
    x1 = poolX.tile([P, NQT, D], F32)
    h2T = poolX.tile([P, MT_D, SQ], F32R)

    with tc.tile_pool(name="w3", bufs=1) as wp3, \
         tc.tile_pool(name="c3", bufs=2) as cp3, \
         tc.tile_pool(name="s3", bufs=4) as sp3, \
         tc.tile_pool(name="ps3", bufs=2, space="PSUM") as ps3, \
         tc.tile_pool(name="ps3b", bufs=3, space="PSUM") as ps3b:
        wot = wp3.tile([64, H, D], F32R)
        nc.sync.dma_start(wot[:], t["wb"].ap()[0:64,
                          _WL["wot"][0]:_WL["wot"][0] + H * D].rearrange(
                          "p (h d) -> p h d", d=D).bitcast(F32R))
        wob_bc = wp3.tile([P, D], F32)
        nc.sync.dma_start(wob_bc[:], wap("wo_b"))
        bo_bc = wp3.tile([P, D], F32)
        nc.sync.dma_start(bo_bc[:], wap("bo"))
        eps3 = wp3.tile([P, 1], F32)
        nc.vector.memset(eps3[:], LN_EPS)

        for tch in range(SQ // TCH3):
            attT = cp3.tile([P, MT_D, TCH3], F32, tag="attT")
            for mt in range(MT_D):
                for n0 in range(0, TCH3, 512):
                    n1 = min(n0 + 512, TCH3)
                    aps = ps3.tile([P, 512], F32, tag="p31")
                    for h in range(H):
                        nc.tensor.matmul(aps[:, 0:n1 - n0],
                                         wot[:, h, mt * P:(mt + 1) * P],
                                         oTn[:, h, tch * TCH3 + n0:tch * TCH3 + n1],
                                         start=(h == 0), stop=(h == H - 1))
                    nc.scalar.activation(attT[:, mt, n0:n1], aps[:, 0:n1 - n0], AF.Copy)
            # stage-batched across the 4 token-blocks: each engine gets a
            # contiguous run of independent work instead of 4 serial
            # PE->DVE->ACT->DVE ping-pong chains
            NTB = TCH3 // P
            mvall = sp3.tile([P, NTB, 2], F32, tag="mv3")
            for tb in range(NTB):
                tt = (tch * TCH3) // P + tb
                tps3 = ps3b.tile([P, D], F32, tag="t3_ps")
                for mt in range(MT_D):
                    nc.tensor.transpose(tps3[:, mt * P:(mt + 1) * P],
                                        attT[:, mt, tb * P:(tb + 1) * P], ident[:])
                xq_t = sp3.tile([P, D], F32, tag="xq_t")
                nc.sync.dma_start(xq_t[:], xblk(tt))
                nc.vector.tensor_tensor(xq_t[:], xq_t[:], wob_bc[:], OP.add)
                nc.vector.tensor_tensor(x1[:, tt, :], tps3[:], xq_t[:], OP.add)
                xg = x1[:, tt, :].rearrange("p (n s) -> p n s", s=256)
                stats = sp3.tile([P, D // 256, 6], F32, tag="st3")
                for g in range(D // 256):
                    nc.vector.bn_stats(stats[:, g, :], xg[:, g, :])
                nc.vector.bn_aggr(mvall[:, tb, :], stats[:])
            rstd4 = sp3.tile([P, NTB], F32, tag="rstd3")
            nc.scalar.activation(rstd4[:], mvall[:, :, 1], AF.Sqrt, bias=eps3[:])
            nc.vector.reciprocal(rstd4[:], rstd4[:])
            h2s = []
            for tb in range(NTB):
                tt = (tch * TCH3) // P + tb
                h2_t = sp3.tile([P, D], F32, tag="h2_t")
                nc.vector.tensor_scalar(h2_t[:], x1[:, tt, :], mvall[:, tb, 0:1],
                                        rstd4[:, tb:tb + 1], OP.subtract, OP.mult)
                nc.vector.tensor_tensor(x1[:, tt, :], x1[:, tt, :], bo_bc[:], OP.add)
                h2s.append((tt, h2_t))
            for tt, h2_t in h2s:
                for mg in range(MT_D // 3):
                    ps = ps3.tile([P, 3, P], F32, tag="p31")
                    for j in range(3):
                        mt = mg * 3 + j
                        nc.tensor.transpose(ps[:, j, :], h2_t[:, mt * P:(mt + 1) * P],
                                            ident[:])
                    nc.scalar.activation(
                        h2T[:, mg * 3:(mg + 1) * 3, tt * P:(tt + 1) * P],
                        ps[:], AF.Copy)
    poolO_cm.__exit__(None, None, None)

    # ---- phase 4: FFN ----
    if _PHASES < 4:
        with tc.tile_pool(name="fb", bufs=2) as fb:
            for tt in range(NQT):
                ft = fb.tile([P, D], F32, tag="ft")
                nc.vector.tensor_copy(ft[:], x1[:, tt, :])
                nc.sync.dma_start(t["out"][tt * P:(tt + 1) * P, :], ft[:])
        poolX_cm.__exit__(None, None, None)
        const_cm.__exit__(None, None, None)
        return
    with tc.tile_pool(name="fw", bufs=1) as fw, \
         tc.tile_pool(name="fs", bufs=2) as fs, \
         tc.tile_pool(name="fcvi", bufs=2) as fcv, \
         tc.tile_pool(name="fc", bufs=2) as fc, \
         tc.tile_pool(name="psU", bufs=2, space="PSUM") as psU, \
         tc.tile_pool(name="psT", bufs=1, space="PSUM") as psT, \
         tc.tile_pool(name="psY", bufs=1, space="PSUM") as psY:
        ui = fw.tile([P, MT_D, RF], F32R)
        nc.sync.dma_start(ui[:], wap("ui").rearrange(
            "p (k m) -> p k m", m=RF).bitcast(F32R))
        vo = fw.tile([P, MT_RF, D], F32R)
        nc.sync.dma_start(vo[:], wap("vo").rearrange(
            "p (k m) -> p k m", m=D).bitcast(F32R))
        bi1 = fw.tile([P, MT_DFF], F32)
        nc.sync.dma_start(bi1[:], wap("bi1t"))
        bi2 = fw.tile([P, MT_DFF], F32)
        nc.sync.dma_start(bi2[:], wap("bi2t"))

        for tch in range(SQ // TCH3):
            NT = TCH3
            w1T = fc.tile([P, MT_RF, NT], F32R, tag="w1T")
            for mt in range(MT_RF):
                for n0 in range(0, NT, 512):
                    n1 = min(n0 + 512, NT)
                    wps = psU.tile([P, 512], F32, tag="ups")
                    for kt in range(MT_D):
                        nc.tensor.matmul(wps[:, 0:n1 - n0],
                                         ui[:, kt, mt * P:(mt + 1) * P],
                                         h2T[:, kt, tch * NT + n0:tch * NT + n1],
                                         start=(kt == 0), stop=(kt == MT_D - 1))
                    nc.scalar.activation(w1T[:, mt, n0:n1], wps[:, 0:n1 - n0], AF.Copy)
            tps = psT.tile([P, MT_RF, 512], F32, tag="t_ps")
            for dch in range(NDCH):
                vi1 = fcv.tile([P, 4, 512], F32R, tag="vi1")
                nc.sync.dma_start(vi1[:], wap("vi", dch * 4096, 2048).rearrange(
                    "p (k m) -> p k m", m=512).bitcast(F32R))
                vi2 = fcv.tile([P, 4, 512], F32R, tag="vi2")
                nc.sync.dma_start(vi2[:], wap("vi", dch * 4096 + 2048, 2048).rearrange(
                    "p (k m) -> p k m", m=512).bitcast(F32R))
                uoc = fcv.tile([P, 4, RF], F32R, tag="uoc")
                nc.sync.dma_start(uoc[:], wap("uo", dch * 2048, 2048).rearrange(
                    "p (k m) -> p k m", m=RF).bitcast(F32R))
                g = fs.tile([P, 4, NT], F32R, tag="g")
                for m4 in range(4):
                    bcol = dch * 4 + m4
                    for n0 in range(0, NT, 512):
                        n1 = min(n0 + 512, NT)
                        u1ps = psU.tile([P, 512], F32, tag="ups")
                        for kt in range(MT_RF):
                            nc.tensor.matmul(u1ps[:, 0:n1 - n0],
                                             vi1[:, kt, m4 * P:(m4 + 1) * P],
                                             w1T[:, kt, n0:n1],
                                             start=(kt == 0), stop=(kt == MT_RF - 1))
                        nc.scalar.activation(g[:, m4, n0:n1], u1ps[:, 0:n1 - n0],
                                             AF.Gelu_apprx_tanh,
                                             bias=bi1[:, bcol:bcol + 1])
                        u2ps = psU.tile([P, 512], F32, tag="ups")
                        for kt in range(MT_RF):
                            nc.tensor.matmul(u2ps[:, 0:n1 - n0],
                                             vi2[:, kt, m4 * P:(m4 + 1) * P],
                                             w1T[:, kt, n0:n1],
                                             start=(kt == 0), stop=(kt == MT_RF - 1))
                        nc.vector.scalar_tensor_tensor(g[:, m4, n0:n1],
                                                       u2ps[:, 0:n1 - n0],
                                                       bi2[:, bcol:bcol + 1],
                                                       g[:, m4, n0:n1],
                                                       OP.add, OP.mult)
                for mr in range(MT_RF):
                    for ktl in range(4):
                        nc.tensor.matmul(tps[:, mr, 0:NT],
                                         uoc[:, ktl, mr * P:(mr + 1) * P],
                                         g[:, ktl, :],
                                         start=(dch == 0 and ktl == 0),
                                         stop=(dch == NDCH - 1 and ktl == 3),
                                         skip_group_check=True)
            tT = fc.tile([P, MT_RF, NT], F32R, tag="tT")
            nc.scalar.activation(tT[:], tps[:, :, 0:NT], AF.Copy)
            yT = fc.tile([P, MT_D, NT], F32, tag="yT")
            for mt in range(MT_D):
                for n0 in range(0, NT, 512):
                    n1 = min(n0 + 512, NT)
                    yps = psU.tile([P, 512], F32, tag="ups")
                    for kt in range(MT_RF):
                        nc.tensor.matmul(yps[:, 0:n1 - n0],
                                         vo[:, kt, mt * P:(mt + 1) * P],
                                         tT[:, kt, n0:n1],
                                         start=(kt == 0), stop=(kt == MT_RF - 1))
                    nc.scalar.activation(yT[:, mt, n0:n1], yps[:, 0:n1 - n0], AF.Copy)
            for tb in range(NT // P):
                tt = (tch * NT) // P + tb
                yps2 = psY.tile([P, D], F32, tag="yt_ps")
                for mt in range(MT_D):
                    nc.tensor.transpose(yps2[:, mt * P:(mt + 1) * P],
                                        yT[:, mt, tb * P:(tb + 1) * P], ident[:])
                o_t = fc.tile([P, D], F32, tag="o_t")
                nc.vector.tensor_tensor(o_t[:], yps2[:], x1[:, tt, :], OP.add)
                nc.sync.dma_start(t["out"][tt * P:(tt + 1) * P, :], o_t[:])
    poolX_cm.__exit__(None, None, None)
    const_cm.__exit__(None, None, None)


def _build_module():
    nc = bacc.Bacc("TRN2", target_bir_lowering=False, debug=False, num_devices=N_CORES)
    t = _declare_io(nc)
    with tile.TileContext(nc) as tc:
        _emit(nc, tc, t)
    nc.compile()
    return nc


def _prep_weights(inputs):
    def rot_last(a):
        return np.concatenate([-a[..., HD // 2:], a[..., :HD // 2]], axis=-1)

    f32 = lambda a: np.ascontiguousarray(np.asarray(a), dtype=np.float32)
    w = {}
    for p, U, V, b in (("q", inputs["Uq"], inputs["Vq"], inputs["bq"]),
                       ("k", inputs["Uk"], inputs["Vk"], inputs["bk"])):
        U, V, b = f32(U), f32(V), f32(b)
        ucat = U.transpose(1, 0, 2).reshape(D, HRA)
        w[f"ucat_{p}"] = ucat.reshape(MT_D, P, HRA).transpose(1, 0, 2).reshape(P, -1)
        for suf, VV in ((p, V), (p + "r", rot_last(V))):
            blk = np.zeros((MT_D, P, P), np.float32)
            for m in range(MT_D):
                for j in range(2):
                    h = 2 * m + j
                    ro = (h % 4) * RA
                    blk[m, ro:ro + RA, 64 * j:64 * j + HD] = VV[h]
            w[f"bdv_{suf}"] = blk.transpose(1, 0, 2).reshape(P, -1)
        w[f"bias_{p}"] = f32(b.reshape(MT_D, P).T)
        w[f"bias_{p}r"] = f32(rot_last(b.reshape(H, HD)).reshape(D).reshape(MT_D, P).T)
    ucv = f32(inputs["Uv"]).transpose(1, 0, 2).reshape(D, HRA)
    w["ucat_v"] = ucv.reshape(MT_D, P, HRA).transpose(1, 0, 2).reshape(P, -1)
    bdvv = np.zeros((HRA, D), np.float32)
    Vv = f32(inputs["Vv"])
    for h in range(H):
        bdvv[h * RA:(h + 1) * RA, h * HD:(h + 1) * HD] = Vv[h]
    w["bdvv"] = bdvv.reshape(KT_A, P, D).transpose(1, 0, 2).reshape(P, -1)
    w["bv"] = np.broadcast_to(f32(inputs["bv"]), (P, D))
    wot = f32(inputs["Wo_w"]).T
    wot64 = np.ascontiguousarray(wot).reshape(H, 64, D).transpose(1, 0, 2).reshape(64, -1)
    w["wot"] = np.concatenate([wot64, np.zeros((64, H * D), np.float32)], 0)
    w["wo_b"] = np.broadcast_to(f32(inputs["Wo_b"]), (P, D))
    w["ui"] = f32(inputs["Ui"]).reshape(MT_D, P, RF).transpose(1, 0, 2).reshape(P, -1)
    vi = f32(inputs["Vi"])
    w["vi"] = vi.reshape(MT_RF, P, 2, NDCH, 512).transpose(
        1, 3, 2, 0, 4).reshape(P, -1)
    bi = f32(inputs["bi"])
    w["bi1t"] = f32(bi[:DFF].reshape(MT_DFF, P).T)
    w["bi2t"] = f32(bi[DFF:].reshape(MT_DFF, P).T)
    uo = f32(inputs["Uo"])
    w["uo"] = uo.reshape(NDCH, MT_RF, P, RF).transpose(2, 0, 1, 3).reshape(P, -1)
    w["vo"] = f32(inputs["Vo"]).reshape(MT_RF, P, D).transpose(1, 0, 2).reshape(P, -1)
    w["bo"] = np.broadcast_to(f32(inputs["bo"]), (P, D))

    wb = np.zeros((P, WCOLS), np.float32)
    for name, (off, n) in _WL.items():
        a = w[name]
        assert a.shape == (P, n), f"{name}: {a.shape} != {(P, n)}"
        wb[:, off:off + n] = a
    return wb


def _make_inmaps(inputs):
    wb = _prep_weights(inputs)
    x = np.asarray(inputs["x"], dtype=np.float32)
    cos = np.asarray(inputs["cos"], dtype=np.float32)
    sin = np.asarray(inputs["sin"], dtype=np.float32)
    in_maps = []
    for core in range(N_CORES):
        b, hf = core // 2, core % 2
        sel = np.r_[hf * SQ:(hf + 1) * SQ, (1 - hf) * SQ:(2 - hf) * SQ]
        xin = np.empty((P, XCOLS), np.float32)
        xo, xn = _XL["xfull"]
        xin[:, xo:xo + xn] = x[b][sel].reshape(NKT, P, D).transpose(1, 0, 2).reshape(P, -1)
        cp, sp = cos[sel].T, sin[sel].T
        co, cn = _XL["cos2"]
        xin[:, co:co + cn] = np.concatenate([cp, cp], 0)
        so, sn = _XL["sin2"]
        xin[:, so:so + sn] = np.concatenate([sp, sp], 0)
        in_maps.append({"xin": xin, "wb": wb})
    return in_maps


def _run(inputs, **kwargs):
    nc = _CACHE.get("nc")
    if nc is None:
        nc = _CACHE["nc"] = _build_module()
    in_maps = _make_inmaps(inputs)
    res = run_bass_kernel_spmd(nc, in_maps, list(range(N_CORES)), **kwargs)
    out = np.empty((B, S, D), np.float32)
    for core in range(N_CORES):
        b, hf = core // 2, core % 2
        out[b, hf * SQ:(hf + 1) * SQ] = res.results[core]["out"]
    return out, res


def kernel(**inputs):
    out, _ = _run(inputs)
    return out


# revision 32
# speedup vs baseline: 1.2568x; 1.2568x over previous
"""Trainium2 Bass kernel for nn_ExplicitSVDBlock (dense transformer block).

Sharding: 8 NeuronCores = 4 batches x 2 query-halves of 1024 tokens.
Each core receives its batch's full 2048 tokens (permuted so its own
query tokens come first), redundantly builds K/V for all keys, and
computes everything else for its 1024 query tokens.  Zero cross-core
communication; host gathers the 8 [1024, 768] shards.

Device program: feature-major activations for matmuls (PE transposes
bridge to token-major for layernorm/residual), float32r matmul dtype,
softmax via exp on ScalarE with a [V | 1]-augmented stationary so the
denominators come out of the same PE accumulation.

I/O: all tensors are packed host-side into two DRAM blobs — a per-core
`xin` ([128, 16384]: x blocks + cos/sin tables) and a core-invariant
weight blob `wb` ([128, 66888]) — so each dispatch binds 2 input
buffers + 1 output instead of 27.  The axon per-dispatch cost is
~26 us per bound input buffer, which dominated the old launch path.
"""
import sys

if '/opt/trn_rl_repo' not in sys.path:
    sys.path.insert(0, '/opt/trn_rl_repo')

import numpy as np
import concourse.bass as bass
import concourse.bacc as bacc
import concourse.mybir as mybir
import concourse.tile as tile
from concourse.bass_utils import run_bass_kernel_spmd
from concourse.masks import make_identity

F32 = mybir.dt.float32
F32R = mybir.dt.float32r
BF16 = mybir.dt.bfloat16
AF = mybir.ActivationFunctionType
OP = mybir.AluOpType

B, S, D, H, HD, RA = 4, 2048, 768, 12, 64, 32
RF, DFF = 512, 3072
P = 128
SK, SQ = S, S // 2          # keys per core / queries per core
HRA = H * RA                # 384
MT_D = D // P               # 6
KT_A = HRA // P             # 3
NKT = SK // P               # 16
NQT = SQ // P               # 8
QCH = 256                   # attention query chunk
NQC = SQ // QCH
KB = 4                      # score k-tiles per exp batch
MT_RF = RF // P             # 4
MT_DFF = DFF // P           # 24
NDCH = DFF // 512           # 6
TCH = 256                   # build token chunk
TCH3 = 512                  # post-attention token chunk
SKH = SK // 2
LN_EPS = 1e-6
N_CORES = 8

# ---- packed input layouts ----
_XL, _WL = {}, {}


def _mk_layout():
    xo = 0
    for name, n in (("xfull", NKT * D), ("cos2", SK), ("sin2", SK)):
        _XL[name] = (xo, n)
        xo += n
    wo = 0
    ents = [("ucat_q", KT_A * HRA * 2), ("ucat_k", KT_A * HRA * 2),
            ("ucat_v", KT_A * HRA * 2),
            ("bdv_q", MT_D * P), ("bdv_qr", MT_D * P),
            ("bdv_k", MT_D * P), ("bdv_kr", MT_D * P),
            ("bias_q", MT_D), ("bias_qr", MT_D),
            ("bias_k", MT_D), ("bias_kr", MT_D),
            ("bdvv", KT_A * D), ("bv", D), ("wot", H * D), ("wo_b", D),
            ("ui", MT_D * RF), ("vi", NDCH * 2 * MT_RF * 512),
            ("bi1t", MT_DFF), ("bi2t", MT_DFF),
            ("uo", NDCH * MT_RF * 512), ("vo", MT_RF * D), ("bo", D)]
    for name, n in ents:
        _WL[name] = (wo, n)
        wo += n
    return xo, wo


XCOLS, WCOLS = _mk_layout()

_CACHE = {}
import os
_PHASES = int(os.environ.get("BASS_KERNEL_PHASES", "4"))
_BCAST_MM = int(os.environ.get("BASS_BCAST_MM", "1"))


def _declare_io(nc):
    t = {}
    t["xin"] = nc.dram_tensor("xin", [P, XCOLS], F32, kind="ExternalInput")
    t["wb"] = nc.dram_tensor("wb", [P, WCOLS], F32, kind="ExternalInput")
    t["out"] = nc.dram_tensor("out", [SQ, D], F32, kind="ExternalOutput")
    if not _BCAST_MM:
        t["nrm"] = nc.dram_tensor("nrm_scratch", [H, NQC, QCH], F32)  # internal
    return t


def _emit(nc, tc, t):
    rsc = float(1.0 / np.sqrt(HD))

    def wap(name, off=0, n=None):
        base, tot = _WL[name]
        if n is None:
            n = tot
        return t["wb"].ap()[:, base + off:base + off + n]

    def xap(name, off=0, n=None):
        base, tot = _XL[name]
        if n is None:
            n = tot
        return t["xin"].ap()[:, base + off:base + off + n]

    def xblk(i):
        # 128-token block i of this core's permuted x, as [P, D]
        return xap("xfull", i * D, D)

    const_cm = tc.tile_pool(name="const", bufs=1)
    const = const_cm.__enter__()
    ident = const.tile([P, P], F32)
    make_identity(nc, ident)
    # one-hot-row stationary for broadcasting the softmax recip row: rows
    # 64..127 are the 64-contract lanes, only lane 0 (partition 64) is 1.0
    bsel = const.tile([P, 64], F32)
    nc.vector.memset(bsel[:], 0.0)
    nc.vector.memset(bsel[HD:HD + 1, :], 1.0)

    poolQKV_cm = tc.tile_pool(name="pQKV", bufs=1)
    poolQKV = poolQKV_cm.__enter__()
    qTr = poolQKV.tile([P, MT_D, SQ], F32R)
    kTr = poolQKV.tile([P, MT_D, SK], F32R)
    vaug = poolQKV.tile([P, NKT, H * (HD + 1)], F32R)
    vaug4 = vaug[:].rearrange("p n (h e) -> p n h e", h=H)

    # ---- phase 1: LN1 + QKV build ----
    with tc.tile_pool(name="bw", bufs=1) as wpool, \
         tc.tile_pool(name="bh", bufs=2) as hpool, \
         tc.tile_pool(name="bxu", bufs=2) as xupool, \
         tc.tile_pool(name="brot", bufs=1) as rotpool, \
         tc.tile_pool(name="bx", bufs=2) as xpool, \
         tc.tile_pool(name="bst", bufs=3) as stpool, \
         tc.tile_pool(name="psA", bufs=3, space="PSUM") as psA, \
         tc.tile_pool(name="psB", bufs=2, space="PSUM") as psB, \
         tc.tile_pool(name="psV", bufs=1, space="PSUM") as psV:

        ucat, bdv, bias = {}, {}, {}
        for p in ("q", "k", "v"):
            w = wpool.tile([P, MT_D, HRA], F32R, tag=f"ucat_{p}")
            nc.sync.dma_start(w[:], wap(f"ucat_{p}").rearrange(
                "p (k m) -> p k m", m=HRA).bitcast(F32R))
            ucat[p] = w
        for p in ("q", "qr", "k", "kr"):
            w = wpool.tile([P, MT_D, P], F32R, tag=f"bdv_{p}")
            nc.sync.dma_start(w[:], wap(f"bdv_{p}").rearrange(
                "p (m x) -> p m x", x=P).bitcast(F32R))
            bdv[p] = w
            bl = wpool.tile([P, MT_D], F32, tag=f"bias_{p}")
            nc.sync.dma_start(bl[:], wap(f"bias_{p}"))
            bias[p] = bl
        bdvv = wpool.tile([P, KT_A, D], F32R)
        nc.sync.dma_start(bdvv[:], wap("bdvv").rearrange(
            "p (k d) -> p k d", d=D).bitcast(F32R))
        bv_bc = wpool.tile([P, D], F32)
        nc.sync.dma_start(bv_bc[:], wap("bv"))
        eps_t = wpool.tile([P, 1], F32)
        nc.vector.memset(eps_t[:], LN_EPS)
        ones_h = wpool.tile([P, H], F32)
        nc.vector.memset(ones_h[:], 1.0)
        for _kt in range(NKT):
            nc.vector.tensor_copy(vaug4[:, _kt, :, HD], ones_h[:])

        for half in range(2):
            goff = half * SKH
            for tch in range(SKH // TCH):
                coff = tch * TCH
                gcoff = goff + coff
                hT = hpool.tile([P, MT_D, TCH], F32R, tag="hT")
                cosc = hpool.tile([P, TCH], F32, tag="cosc")
                sinc = hpool.tile([P, TCH], F32, tag="sinc")
                nc.sync.dma_start(cosc[:], xap("cos2", gcoff, TCH))
                nc.sync.dma_start(sinc[:], xap("sin2", gcoff, TCH))

                for tb in range(TCH // P):
                    x_t = xpool.tile([P, D], F32, tag="x_t")
                    r0 = gcoff + tb * P
                    nc.sync.dma_start(x_t[:], xblk(r0 // P))
                    xg = x_t[:].rearrange("p (n s) -> p n s", s=256)
                    stats = stpool.tile([P, D // 256, 6], F32, tag="stats")
                    for g in range(D // 256):
                        nc.vector.bn_stats(stats[:, g, :], xg[:, g, :])
                    mv = stpool.tile([P, 2], F32, tag="mv")
                    nc.vector.bn_aggr(mv[:], stats[:])
                    rstd = stpool.tile([P, 1], F32, tag="rstd")
                    nc.scalar.activation(rstd[:], mv[:, 1:2], AF.Sqrt, bias=eps_t[:])
                    nc.vector.reciprocal(rstd[:], rstd[:])
                    nc.vector.tensor_scalar(x_t[:], x_t[:], mv[:, 0:1], rstd[:],
                                            OP.subtract, OP.mult)
                    for mg in range(MT_D // 3):
                        ps = psA.tile([P, 3, P], F32, tag="b1")
                        for j in range(3):
                            mt = mg * 3 + j
                            nc.tensor.transpose(ps[:, j, :],
                                                x_t[:, mt * P:(mt + 1) * P], ident[:])
                        nc.scalar.activation(
                            hT[:, mg * 3:(mg + 1) * 3, tb * P:(tb + 1) * P],
                            ps[:], AF.Copy)

                projs = ["k", "v"] + (["q"] if half == 0 else [])
                for p in projs:
                    xs = xupool.tile([P, KT_A, TCH], F32R, tag="xu_sb")
                    for ma in range(KT_A):
                        xps = psA.tile([P, TCH], F32, tag="b1")
                        for kt in range(MT_D):
                            nc.tensor.matmul(xps[:], ucat[p][:, kt, ma * P:(ma + 1) * P],
                                             hT[:, kt, :],
                                             start=(kt == 0), stop=(kt == MT_D - 1))
                        nc.scalar.activation(xs[:, ma, :], xps[:], AF.Copy)
                    if p == "v":
                        for tb in range(TCH // P):
                            vps = psV.tile([P, D], F32, tag="v_ps")
                            for n0 in range(0, D, 512):
                                n1 = min(n0 + 512, D)
                                for ka in range(KT_A):
                                    nc.tensor.matmul(vps[:, n0:n1],
                                                     xs[:, ka, tb * P:(tb + 1) * P],
                                                     bdvv[:, ka, n0:n1],
                                                     start=(ka == 0),
                                                     stop=(ka == KT_A - 1))
                            ktg = gcoff // P + tb
                            nc.vector.tensor_tensor(
                                vaug4[:, ktg, :, 0:HD],
                                vps[:].rearrange("p (h e) -> p h e", h=H),
                                bv_bc[:].rearrange("p (h e) -> p h e", h=H),
                                OP.add)
                    else:
                        dst = qTr if p == "q" else kTr
                        dcols = slice(coff, coff + TCH) if p == "q" else \
                                slice(gcoff, gcoff + TCH)
                        rot = rotpool.tile([P, MT_D, TCH], F32R, tag="rot")
                        for m in range(MT_D):
                            ps2 = psB.tile([P, TCH], F32, tag="st2")
                            nc.tensor.matmul(ps2[:], bdv[p][:, m, :], xs[:, m // 2, :],
                                             start=True, stop=True)
                            nc.scalar.activation(dst[:, m, dcols], ps2[:], AF.Identity,
                                                 bias=bias[p][:, m:m + 1])
                            ps3 = psB.tile([P, TCH], F32, tag="st2")
                            nc.tensor.matmul(ps3[:], bdv[p + "r"][:, m, :],
                                             xs[:, m // 2, :], start=True, stop=True)
                            nc.vector.scalar_tensor_tensor(
                                rot[:, m, :], ps3[:], bias[p + "r"][:, m:m + 1],
                                sinc[:], OP.add, OP.mult)
                        dsl = dst[:, :, dcols]
                        cb = cosc[:, None, :].to_broadcast([P, MT_D, TCH])
                        nc.vector.tensor_tensor(dsl, dsl, cb, OP.mult)
                        nc.vector.tensor_tensor(dsl, dsl, rot[:], OP.add)

    # ---- phase 2: attention ----
    if _PHASES < 2:
        poolQKV_cm.__exit__(None, None, None)
        with tc.tile_pool(name="fb", bufs=2) as fb:
            for tt in range(NQT):
                ft = fb.tile([P, D], F32, tag="ft")
                nc.sync.dma_start(ft[:], xblk(tt))
                nc.sync.dma_start(t["out"][tt * P:(tt + 1) * P, :], ft[:])
        const_cm.__exit__(None, None, None)
        return
    poolO_cm = tc.tile_pool(name="pO", bufs=1, side="right")
    poolO = poolO_cm.__enter__()
    oTn = poolO.tile([64, H, SQ], F32R)
    if _BCAST_MM:
        srow_p = poolO.tile([P, QCH], F32)
        nc.vector.memset(srow_p[:], 0.0)

    with tc.tile_pool(name="aexp", bufs=2, side="right") as apool, \
         tc.tile_pool(name="anrm", bufs=3, side="right") as npool, \
         tc.tile_pool(name="psS", bufs=3, space="PSUM") as psS, \
         tc.tile_pool(name="psO", bufs=2, space="PSUM") as psO:
        items = [(h, qc) for h in range(H) for qc in range(NQC)]

        def emit_scores(i):
            h, qc = items[i]
            pair, hh = h // 2, h % 2
            rs = slice(hh * 64, hh * 64 + 64)
            qcols = slice(qc * QCH, (qc + 1) * QCH)
            expS = apool.tile([P, NKT, QCH], F32R, tag="expS")
            for kb in range(NKT // KB):
                sps = psS.tile([P, KB, QCH], F32, tag="s_ps")
                for j in range(KB):
                    kt = kb * KB + j
                    nc.tensor.matmul(sps[:, j, :],
                                     kTr[rs, pair, kt * P:(kt + 1) * P],
                                     qTr[rs, pair, qcols],
                                     start=True, stop=True)
                nc.scalar.activation(expS[:, kb * KB:(kb + 1) * KB, :],
                                     sps[:], AF.Exp, scale=rsc)
            return expS

        def emit_av(i, expS):
            h, qc = items[i]
            qcols = slice(qc * QCH, (qc + 1) * QCH)
            po = psO.tile([P, QCH], F32, tag="o_ps")
            for kt in range(NKT):
                nc.tensor.matmul(po[0:HD + 1, :], vaug4[:, kt, h, :],
                                 expS[:, kt, :],
                                 start=(kt == 0), stop=(kt == NKT - 1))
            if _BCAST_MM:
                nc.vector.reciprocal(srow_p[HD:HD + 1, :], po[HD:HD + 1, :])
                # broadcast the recip row across the 64 o-lanes on the PE
                # (one-hot-row stationary) instead of a DRAM roundtrip
                rb = psS.tile([P, KB, QCH], F32, tag="s_ps")
                nc.tensor.matmul(rb[0:64, 0, :], bsel[HD:, :], srow_p[HD:, :],
                                 start=True, stop=True)
                # DVE cannot read two PSUM operands; stage rb through SBUF
                rbs = npool.tile([64, QCH], F32, tag="rbs")
                nc.vector.tensor_copy(rbs[:], rb[0:64, 0, :])
                nc.vector.tensor_tensor(oTn[:, h, qcols], po[0:HD, :],
                                        rbs[:], OP.mult)
            else:
                srow = npool.tile([P, QCH], F32, tag="srow")
                nc.vector.reciprocal(srow[HD:HD + 1, :], po[HD:HD + 1, :])
                nc.sync.dma_start(t["nrm"][h, qc, :], srow[HD:HD + 1, :])
                rbd = npool.tile([64, QCH], F32, tag="rb")
                nc.gpsimd.dma_start(
                    rbd[:], bass.AP(t["nrm"].ap().tensor,
                                    (h * NQC + qc) * QCH, [[0, 64], [1, QCH]]))
                nc.vector.tensor_tensor(oTn[:, h, qcols], po[0:HD, :],
                                        rbd[:], OP.mult)

        # software pipeline: scores for item i+1 overlap AV/normalize of item i
        prev = None
        for i in range(len(items) + 1):
            cur = emit_scores(i) if i < len(items) else None
            if prev is not None:
                emit_av(i - 1, prev)
            prev = cur
    poolQKV_cm.__exit__(None, None, None)

    # ---- phase 3: Wo + residual + LN2 ----
    if _PHASES < 3:
        poolO_cm.__exit__(None, None, None)
        with tc.tile_pool(name="fb", bufs=2) as fb:
            for tt in range(NQT):
                ft = fb.tile([P, D], F32, tag="ft")
                nc.sync.dma_start(ft[:], xblk(tt))
                nc.sync.dma_start(t["out"][tt * P:(tt + 1) * P, :], ft[:])
        const_cm.__exit__(None, None, None)
        return
    poolX_cm = tc.tile_pool(name="pX", bufs=1)
    poolX = poolX_cm.__enter__()
    x1 = poolX.tile([P, NQT, D], F32)
    h2T = poolX.tile([P, MT_D, SQ], F32R)

    with tc.tile_pool(name="w3", bufs=1) as wp3, \
         tc.tile_pool(name="c3", bufs=2) as cp3, \
         tc.tile_pool(name="s3", bufs=4) as sp3, \
         tc.tile_pool(name="ps3", bufs=2, space="PSUM") as ps3, \
         tc.tile_pool(name="ps3b", bufs=3, space="PSUM") as ps3b:
        wot = wp3.tile([64, H, D], F32R)
        nc.sync.dma_start(wot[:], t["wb"].ap()[0:64,
                          _WL["wot"][0]:_WL["wot"][0] + H * D].rearrange(
                          "p (h d) -> p h d", d=D).bitcast(F32R))
        wob_bc = wp3.tile([P, D], F32)
        nc.sync.dma_start(wob_bc[:], wap("wo_b"))
        bo_bc = wp3.tile([P, D], F32)
        nc.sync.dma_start(bo_bc[:], wap("bo"))
        eps3 = wp3.tile([P, 1], F32)
        nc.vector.memset(eps3[:], LN_EPS)

        for tch in range(SQ // TCH3):
            attT = cp3.tile([P, MT_D, TCH3], F32, tag="attT")
            for mt in range(MT_D):
                for n0 in range(0, TCH3, 512):
                    n1 = min(n0 + 512, TCH3)
                    aps = ps3.tile([P, 512], F32, tag="p31")
                    for h in range(H):
                        nc.tensor.matmul(aps[:, 0:n1 - n0],
                                         wot[:, h, mt * P:(mt + 1) * P],
                                         oTn[:, h, tch * TCH3 + n0:tch * TCH3 + n1],
                                         start=(h == 0), stop=(h == H - 1))
                    nc.scalar.activation(attT[:, mt, n0:n1], aps[:, 0:n1 - n0], AF.Copy)
            for tb in range(TCH3 // P):
                tt = (tch * TCH3) // P + tb
                tps3 = ps3b.tile([P, D], F32, tag="t3_ps")
                for mt in range(MT_D):
                    nc.tensor.transpose(tps3[:, mt * P:(mt + 1) * P],
                                        attT[:, mt, tb * P:(tb + 1) * P], ident[:])
                xq_t = sp3.tile([P, D], F32, tag="xq_t")
                nc.sync.dma_start(xq_t[:], xblk(tt))
                nc.vector.tensor_tensor(xq_t[:], xq_t[:], wob_bc[:], OP.add)
                nc.vector.tensor_tensor(x1[:, tt, :], tps3[:], xq_t[:], OP.add)
                xg = x1[:, tt, :].rearrange("p (n s) -> p n s", s=256)
                stats = sp3.tile([P, D // 256, 6], F32, tag="st3")
                for g in range(D // 256):
                    nc.vector.bn_stats(stats[:, g, :], xg[:, g, :])
                mv = sp3.tile([P, 2], F32, tag="mv3")
                nc.vector.bn_aggr(mv[:], stats[:])
                rstd = sp3.tile([P, 1], F32, tag="rstd3")
                nc.scalar.activation(rstd[:], mv[:, 1:2], AF.Sqrt, bias=eps3[:])
                nc.vector.reciprocal(rstd[:], rstd[:])
                h2_t = sp3.tile([P, D], F32, tag="h2_t")
                nc.vector.tensor_scalar(h2_t[:], x1[:, tt, :], mv[:, 0:1], rstd[:],
                                        OP.subtract, OP.mult)
                nc.vector.tensor_tensor(x1[:, tt, :], x1[:, tt, :], bo_bc[:], OP.add)
                for mg in range(MT_D // 3):
                    ps = ps3.tile([P, 3, P], F32, tag="p31")
                    for j in range(3):
                        mt = mg * 3 + j
                        nc.tensor.transpose(ps[:, j, :], h2_t[:, mt * P:(mt + 1) * P],
                                            ident[:])
                    nc.scalar.activation(
                        h2T[:, mg * 3:(mg + 1) * 3, tt * P:(tt + 1) * P],
                        ps[:], AF.Copy)
    poolO_cm.__exit__(None, None, None)

    # ---- phase 4: FFN ----
    if _PHASES < 4:
        with tc.tile_pool(name="fb", bufs=2) as fb:
            for tt in range(NQT):
                ft = fb.tile([P, D], F32, tag="ft")
                nc.vector.tensor_copy(ft[:], x1[:, tt, :])
                nc.sync.dma_start(t["out"][tt * P:(tt + 1) * P, :], ft[:])
        poolX_cm.__exit__(None, None, None)
        const_cm.__exit__(None, None, None)
        return
    with tc.tile_pool(name="fw", bufs=1) as fw, \
         tc.tile_pool(name="fs", bufs=2) as fs, \
         tc.tile_pool(name="fcvi", bufs=2) as fcv, \
         tc.tile_pool(name="fc", bufs=2) as fc, \
         tc.tile_pool(name="psU", bufs=2, space="PSUM") as psU, \
         tc.tile_pool(name="psT", bufs=1, space="PSUM") as psT, \
         tc.tile_pool(name="psY", bufs=1, space="PSUM") as psY:
        ui = fw.tile([P, MT_D, RF], F32R)
        nc.sync.dma_start(ui[:], wap("ui").rearrange(
            "p (k m) -> p k m", m=RF).bitcast(F32R))
        vo = fw.tile([P, MT_RF, D], F32R)
        nc.sync.dma_start(vo[:], wap("vo").rearrange(
            "p (k m) -> p k m", m=D).bitcast(F32R))
        bi1 = fw.tile([P, MT_DFF], F32)
        nc.sync.dma_start(bi1[:], wap("bi1t"))
        bi2 = fw.tile([P, MT_DFF], F32)
        nc.sync.dma_start(bi2[:], wap("bi2t"))

        for tch in range(SQ // TCH3):
            NT = TCH3
            w1T = fc.tile([P, MT_RF, NT], F32R, tag="w1T")
            for mt in range(MT_RF):
                for n0 in range(0, NT, 512):
                    n1 = min(n0 + 512, NT)
                    wps = psU.tile([P, 512], F32, tag="ups")
                    for kt in range(MT_D):
                        nc.tensor.matmul(wps[:, 0:n1 - n0],
                                         ui[:, kt, mt * P:(mt + 1) * P],
                                         h2T[:, kt, tch * NT + n0:tch * NT + n1],
                                         start=(kt == 0), stop=(kt == MT_D - 1))
                    nc.scalar.activation(w1T[:, mt, n0:n1], wps[:, 0:n1 - n0], AF.Copy)
            tps = psT.tile([P, MT_RF, 512], F32, tag="t_ps")
            for dch in range(NDCH):
                vi1 = fcv.tile([P, 4, 512], F32R, tag="vi1")
                nc.sync.dma_start(vi1[:], wap("vi", dch * 4096, 2048).rearrange(
                    "p (k m) -> p k m", m=512).bitcast(F32R))
                vi2 = fcv.tile([P, 4, 512], F32R, tag="vi2")
                nc.sync.dma_start(vi2[:], wap("vi", dch * 4096 + 2048, 2048).rearrange(
                    "p (k m) -> p k m", m=512).bitcast(F32R))
                uoc = fcv.tile([P, 4, RF], F32R, tag="uoc")
                nc.sync.dma_start(uoc[:], wap("uo", dch * 2048, 2048).rearrange(
                    "p (k m) -> p k m", m=RF).bitcast(F32R))
                g = fs.tile([P, 4, NT], F32R, tag="g")
                for m4 in range(4):
                    bcol = dch * 4 + m4
                    for n0 in range(0, NT, 512):
                        n1 = min(n0 + 512, NT)
                        u1ps = psU.tile([P, 512], F32, tag="ups")
                        for kt in range(MT_RF):
                            nc.tensor.matmul(u1ps[:, 0:n1 - n0],
                                             vi1[:, kt, m4 * P:(m4 + 1) * P],
                                             w1T[:, kt, n0:n1],
                                             start=(kt == 0), stop=(kt == MT_RF - 1))
                        nc.scalar.activation(g[:, m4, n0:n1], u1ps[:, 0:n1 - n0],
                                             AF.Gelu_apprx_tanh,
                                             bias=bi1[:, bcol:bcol + 1])
                        u2ps = psU.tile([P, 512], F32, tag="ups")
                        for kt in range(MT_RF):
                            nc.tensor.matmul(u2ps[:, 0:n1 - n0],
                                             vi2[:, kt, m4 * P:(m4 + 1) * P],
                                             w1T[:, kt, n0:n1],
                                             start=(kt == 0), stop=(kt == MT_RF - 1))
                        nc.vector.scalar_tensor_tensor(g[:, m4, n0:n1],
                                                       u2ps[:, 0:n1 - n0],
                                                       bi2[:, bcol:bcol + 1],
                                                       g[:, m4, n0:n1],
                                                       OP.add, OP.mult)
                for mr in range(MT_RF):
                    for ktl in range(4):
                        nc.tensor.matmul(tps[:, mr, 0:NT],
                                         uoc[:, ktl, mr * P:(mr + 1) * P],
                                         g[:, ktl, :],
                                         start=(dch == 0 and ktl == 0),
                                         stop=(dch == NDCH - 1 and ktl == 3),
                                         skip_group_check=True)
            tT = fc.tile([P, MT_RF, NT], F32R, tag="tT")
            nc.scalar.activation(tT[:], tps[:, :, 0:NT], AF.Copy)
            yT = fc.tile([P, MT_D, NT], F32, tag="yT")
            for mt in range(MT_D):
                for n0 in range(0, NT, 512):
                    n1 = min(n0 + 512, NT)
                    yps = psU.tile([P, 512], F32, tag="ups")
                    for kt in range(MT_RF):
                        nc.tensor.matmul(yps[:, 0:n1 - n0],
                                         vo[:, kt, mt * P:(mt + 1) * P],
                                         tT[:, kt, n0:n1],
                                         start=(kt == 0), stop=(kt == MT_RF - 1))
                    nc.scalar.activation(yT[:, mt, n0:n1], yps[:, 0:n1 - n0], AF.Copy)
            for tb in range(NT // P):
                tt = (tch * NT) // P + tb
                yps2 = psY.tile([P, D], F32, tag="yt_ps")
                for mt in range(MT_D):
                    nc.tensor.transpose(yps2[:, mt * P:(mt + 1) * P],
                                        yT[:, mt, tb * P:(tb + 1) * P], ident[:])
                o_t = fc.tile([P, D], F32, tag="o_t")
                nc.vector.tensor_tensor(o_t[:], yps2[:], x1[:, tt, :], OP.add)
                nc.sync.dma_start(t["out"][tt * P:(tt + 1) * P, :], o_t[:])
    poolX_cm.__exit__(None, None, None)
    const_cm.__exit__(None, None, None)


def _build_module():
    nc = bacc.Bacc("TRN2", target_bir_lowering=False, debug=False, num_devices=N_CORES)
    t = _declare_io(nc)
    with tile.TileContext(nc) as tc:
        _emit(nc, tc, t)
    nc.compile()
    return nc


def _prep_weights(inputs):
    def rot_last(a):
        return np.concatenate([-a[..., HD // 2:], a[..., :HD // 2]], axis=-1)

    f32 = lambda a: np.ascontiguousarray(np.asarray(a), dtype=np.float32)
    w = {}
    for p, U, V, b in (("q", inputs["Uq"], inputs["Vq"], inputs["bq"]),
                       ("k", inputs["Uk"], inputs["Vk"], inputs["bk"])):
        U, V, b = f32(U), f32(V), f32(b)
        ucat = U.transpose(1, 0, 2).reshape(D, HRA)
        w[f"ucat_{p}"] = ucat.reshape(MT_D, P, HRA).transpose(1, 0, 2).reshape(P, -1)
        for suf, VV in ((p, V), (p + "r", rot_last(V))):
            blk = np.zeros((MT_D, P, P), np.float32)
            for m in range(MT_D):
                for j in range(2):
                    h = 2 * m + j
                    ro = (h % 4) * RA
                    blk[m, ro:ro + RA, 64 * j:64 * j + HD] = VV[h]
            w[f"bdv_{suf}"] = blk.transpose(1, 0, 2).reshape(P, -1)
        w[f"bias_{p}"] = f32(b.reshape(MT_D, P).T)
        w[f"bias_{p}r"] = f32(rot_last(b.reshape(H, HD)).reshape(D).reshape(MT_D, P).T)
    ucv = f32(inputs["Uv"]).transpose(1, 0, 2).reshape(D, HRA)
    w["ucat_v"] = ucv.reshape(MT_D, P, HRA).transpose(1, 0, 2).reshape(P, -1)
    bdvv = np.zeros((HRA, D), np.float32)
    Vv = f32(inputs["Vv"])
    for h in range(H):
        bdvv[h * RA:(h + 1) * RA, h * HD:(h + 1) * HD] = Vv[h]
    w["bdvv"] = bdvv.reshape(KT_A, P, D).transpose(1, 0, 2).reshape(P, -1)
    w["bv"] = np.broadcast_to(f32(inputs["bv"]), (P, D))
    wot = f32(inputs["Wo_w"]).T
    wot64 = np.ascontiguousarray(wot).reshape(H, 64, D).transpose(1, 0, 2).reshape(64, -1)
    w["wot"] = np.concatenate([wot64, np.zeros((64, H * D), np.float32)], 0)
    w["wo_b"] = np.broadcast_to(f32(inputs["Wo_b"]), (P, D))
    w["ui"] = f32(inputs["Ui"]).reshape(MT_D, P, RF).transpose(1, 0, 2).reshape(P, -1)
    vi = f32(inputs["Vi"])
    w["vi"] = vi.reshape(MT_RF, P, 2, NDCH, 512).transpose(
        1, 3, 2, 0, 4).reshape(P, -1)
    bi = f32(inputs["bi"])
    w["bi1t"] = f32(bi[:DFF].reshape(MT_DFF, P).T)
    w["bi2t"] = f32(bi[DFF:].reshape(MT_DFF, P).T)
    uo = f32(inputs["Uo"])
    w["uo"] = uo.reshape(NDCH, MT_RF, P, RF).transpose(2, 0, 1, 3).reshape(P, -1)
    w["vo"] = f32(inputs["Vo"]).reshape(MT_RF, P, D).transpose(1, 0, 2).reshape(P, -1)
    w["bo"] = np.broadcast_to(f32(inputs["bo"]), (P, D))

    wb = np.zeros((P, WCOLS), np.float32)
    for name, (off, n) in _WL.items():
        a = w[name]
        assert a.shape == (P, n), f"{name}: {a.shape} != {(P, n)}"
        wb[:, off:off + n] = a
    return wb


def _make_inmaps(inputs):
    wb = _prep_weights(inputs)
    x = np.asarray(inputs["x"], dtype=np.float32)
    cos = np.asarray(inputs["cos"], dtype=np.float32)
    sin = np.asarray(inputs["sin"], dtype=np.float32)
    in_maps = []
    for core in range(N_CORES):
        b, hf = core // 2, core % 2
        sel = np.r_[hf * SQ:(hf + 1) * SQ, (1 - hf) * SQ:(2 - hf) * SQ]
        xin = np.empty((P, XCOLS), np.float32)
        xo, xn = _XL["xfull"]
        xin[:, xo:xo + xn] = x[b][sel].reshape(NKT, P, D).transpose(1, 0, 2).reshape(P, -1)
        cp, sp = cos[sel].T, sin[sel].T
        co, cn = _XL["cos2"]
        xin[:, co:co + cn] = np.concatenate([cp, cp], 0)
        so, sn = _XL["sin2"]
        xin[:, so:so + sn] = np.concatenate([sp, sp], 0)
        in_maps.append({"xin": xin, "wb": wb})
    return in_maps


def _run(inputs, **kwargs):
    nc = _CACHE.get("nc")
    if nc is None:
        nc = _CACHE["nc"] = _build_module()
    in_maps = _make_inmaps(inputs)
    res = run_bass_kernel_spmd(nc, in_maps, list(range(N_CORES)), **kwargs)
    out = np.empty((B, S, D), np.float32)
    for core in range(N_CORES):
        b, hf = core // 2, core % 2
        out[b, hf * SQ:(hf + 1) * SQ] = res.results[core]["out"]
    return out, res


def kernel(**inputs):
    out, _ = _run(inputs)
    return out
